# revision 10
# baseline (speedup 1.0000x reference)
"""GCN classifier on 8 Trainium2 NeuronCores (Bass/Tile).

Pipeline (SPMD, one program, per-core data):
  phase A : h1 = x @ W1 per node tile (PE), streamed to DRAM slice,
            chunked AllGather -> table1 [TOT, 64]
  layer 1 : per dst tile: dma_gather rows of table1 by edge src,
            sel[e,d] = norm_e * (dstlocal_e == d)  (one DVE tensor_scalar),
            psum[64,128] += msg^T @ sel  (PE, transposed-out),
            epilogue relu(+b1) (ACT) -> o1T, h2 = o1T^T @ W2 (PE) -> DRAM,
            chunked AllGather -> table2
  layer 2 : same gather/sel, normal-out psum[128,64] += sel^T @ msg,
            epilogue +b2, relu, pooled psum += onehotG^T @ o2 (PE)
  phase C : pooled/cnt, transpose, logits = pooledT^T @ Wl + bl -> out [GPC,10]

Sharding: G/8 consecutive graphs per core (batch is sorted so node ranges are
contiguous; pooling is local). Edges assigned to the core owning their dst.
The normalization norm_e = dinv[src]*dinv[dst] rides entirely in the
selection matrix, so tables hold plain h and epilogues are just bias+relu.
"""

import math
from contextlib import ExitStack

import numpy as np

NC_CORES = 8
DIN = 128
H = 64
C = 10
P = 128
WIN = 32768  # int16 index window (rows) for dma_gather
GRP = 6      # dst tiles per gather group
NCHUNK = 4   # allgather chunks per layer


# ---------------------------------------------------------------- preprocess

def _preprocess(x, edge_index, batch, win=WIN, grp=GRP, nchunk=NCHUNK,
                force_g=None):
    x = np.asarray(x, np.float32)
    edge_index = np.asarray(edge_index)
    batch = np.asarray(batch).astype(np.int64)
    N = x.shape[0]
    E = edge_index.shape[1]
    ng = int(batch.max()) + 1 if N else 1
    if force_g is not None:
        ng = max(ng, force_g)
    G = NC_CORES * int(math.ceil(ng / NC_CORES))
    GPC = G // NC_CORES
    assert GPC <= P

    src = np.concatenate([edge_index[0], np.arange(N, dtype=edge_index.dtype)])
    dst = np.concatenate([edge_index[1], np.arange(N, dtype=edge_index.dtype)])
    src = src.astype(np.int64)
    dst = dst.astype(np.int64)

    deg = np.bincount(dst, minlength=N).astype(np.float32)
    dinv = np.where(deg > 0, 1.0 / np.sqrt(deg), 0.0).astype(np.float32)
    norm = (dinv[src] * dinv[dst]).astype(np.float32)

    # core boundaries: GPC consecutive graphs per core
    gstart = np.searchsorted(batch, np.arange(0, G + 1, GPC)).astype(np.int64)
    cnt = np.diff(gstart)
    NMAX = int(math.ceil(max(int(cnt.max()), 1) / P) * P)
    NT = NMAX // P

    # allgather chunking by tile ranges
    tpc = int(math.ceil(NT / nchunk))
    chunk_t0 = [k * tpc for k in range(nchunk) if k * tpc < NT]
    nchunk = len(chunk_t0)
    chunk_t1 = [min(t0 + tpc, NT) for t0 in chunk_t0]
    chrows = [P * (t1 - t0) for t0, t1 in zip(chunk_t0, chunk_t1)]
    chbase = np.concatenate([[0], np.cumsum([NC_CORES * r for r in chrows])])
    TOT = int(chbase[-1])
    chunk_of_tile = np.zeros(NT, np.int64)
    for k, (t0, t1) in enumerate(zip(chunk_t0, chunk_t1)):
        chunk_of_tile[t0:t1] = k
    chunk_t0_a = np.asarray(chunk_t0, np.int64)
    chrows_a = np.asarray(chrows, np.int64)

    # table row of each node (in chunked-allgather layout)
    nodes = np.arange(N, dtype=np.int64)
    ncore = np.minimum(batch // GPC, NC_CORES - 1)
    local = nodes - gstart[ncore]
    ntile = local >> 7
    nk = chunk_of_tile[ntile]
    trow = chbase[nk] + ncore * chrows_a[nk] + (local - P * chunk_t0_a[nk])

    NW = int(math.ceil(TOT / win))

    # per-edge placement
    ecore = np.minimum(batch[dst] // GPC, NC_CORES - 1)
    dl = dst - gstart[ecore]
    et = dl >> 7
    edl = (dl & 127).astype(np.float32)
    erow = trow[src]
    ew = erow // win
    elidx = (erow % win).astype(np.int16)

    M = E + N
    key = (ecore * NT + et) * NW + ew
    cnts = np.bincount(key, minlength=NC_CORES * NT * NW).reshape(
        NC_CORES, NT, NW)
    Btw = np.ceil(cnts.max(axis=0) / P).astype(np.int64)  # [NT, NW]

    # block/gather layout (shared across cores)
    groups = [list(range(g0, min(g0 + grp, NT))) for g0 in range(0, NT, grp)]
    slotoff = np.zeros((NT, NW), np.int64)
    blocks = []   # (t, w, chunk_index) in program order
    gathers = []  # dicts: gi, w, num, chunk0
    grp_ch0 = []  # first chunk of each group
    ch = 0
    for gi, g in enumerate(groups):
        grp_ch0.append(ch)
        for w in range(NW):
            niw = 0
            for t in g:
                slotoff[t, w] = ch * P
                for _ in range(int(Btw[t, w])):
                    blocks.append((t, w, ch))
                    ch += 1
                niw += int(Btw[t, w]) * P
            if niw > 0:
                gathers.append(dict(gi=gi, w=w, num=niw, chunk0=ch - niw // P))
    NBLK = ch
    SLOTS = NBLK * P
    IDXCOLS = SLOTS // 16

    # rank of each edge within its (core, tile, window) segment
    order = np.lexsort((erow, ew, et, ecore))
    skey = key[order]
    seg_change = np.empty(M, bool)
    seg_change[0] = True
    seg_change[1:] = skey[1:] != skey[:-1]
    seg_id = np.cumsum(seg_change) - 1
    seg_first = np.where(seg_change)[0]
    rank_sorted = np.arange(M) - seg_first[seg_id]
    rank = np.empty(M, np.int64)
    rank[order] = rank_sorted

    slot = slotoff[et, ew] + rank

    gidx_slots = np.zeros((NC_CORES, SLOTS), np.int16)
    dsl = np.full((NC_CORES, SLOTS), -1.0, np.float32)
    nrm = np.zeros((NC_CORES, SLOTS), np.float32)
    gidx_slots[ecore, slot] = elidx
    dsl[ecore, slot] = edl
    nrm[ecore, slot] = norm

    # [core, 128, NBLK] with block b in column b
    dstloc_arr = np.ascontiguousarray(
        dsl.reshape(NC_CORES, NBLK, P).transpose(0, 2, 1))
    nrm_arr = np.ascontiguousarray(
        nrm.reshape(NC_CORES, NBLK, P).transpose(0, 2, 1))

    # idx stream packed (i%16, i//16) relative to each 128-aligned segment,
    # replicated into all 8 16-partition groups (one per Q7 core on HW)
    gp = gidx_slots.reshape(NC_CORES, IDXCOLS, 16).transpose(0, 2, 1)
    gidx_packed = np.ascontiguousarray(np.tile(gp, (1, 8, 1)))

    # batch-local graph id per node [core, 128, NT], -1 for pad
    batchloc = np.full((NC_CORES, P, NT), -1.0, np.float32)
    counts = np.bincount(batch, minlength=G).astype(np.float32)
    invcnt = np.ones((NC_CORES, P, 1), np.float32)
    xt = np.zeros((NC_CORES, DIN, NMAX), np.float32)
    for c in range(NC_CORES):
        n0, n1 = int(gstart[c]), int(gstart[c + 1])
        full = np.full(NMAX, -1.0, np.float32)
        full[: n1 - n0] = (batch[n0:n1] - c * GPC).astype(np.float32)
        batchloc[c] = full.reshape(NT, P).T
        invcnt[c, :GPC, 0] = 1.0 / np.maximum(
            counts[c * GPC:(c + 1) * GPC], 1.0)
        xt[c, :, : n1 - n0] = x[n0:n1].T

    meta = dict(
        NT=NT, NW=NW, NMAX=NMAX, TOT=TOT, WIN=win, NBLK=NBLK,
        IDXCOLS=IDXCOLS, GPC=GPC, G=G, GRP=grp,
        groups=groups, gathers=gathers, blocks=blocks, grp_ch0=grp_ch0,
        chunk_t0=chunk_t0, chunk_t1=chunk_t1, chrows=chrows,
        chbase=[int(v) for v in chbase], nchunk=nchunk,
    )
    percore = dict(
        xt=xt, gidx=gidx_packed, dstloc=dstloc_arr, nrm=nrm_arr,
        batchloc=batchloc, invcnt=invcnt,
    )
    return meta, percore


def _weights_inputs(W1, b1, W2, b2, Wl, bl):
    W1 = np.asarray(W1, np.float32)
    W2 = np.asarray(W2, np.float32)
    Wl = np.asarray(Wl, np.float32)
    b1c = np.ascontiguousarray(np.asarray(b1, np.float32).reshape(H, 1))
    b2t = np.ascontiguousarray(
        np.broadcast_to(np.asarray(b2, np.float32), (P, H)))
    blt = np.ascontiguousarray(
        np.broadcast_to(np.asarray(bl, np.float32), (P, C)))
    iota = np.ascontiguousarray(
        np.broadcast_to(np.arange(P, dtype=np.float32), (P, P)))
    return dict(w1=W1, w2=W2, wl=Wl, b1=b1c, b2t=b2t, blt=blt, iota=iota)


# ------------------------------------------------------------------ builder

def _build(tc, outs, ins, meta):
    import concourse.bass as bass
    import concourse.mybir as mybir
    from concourse.masks import make_identity

    nc = tc.nc
    NT, NW, TOT = meta["NT"], meta["NW"], meta["TOT"]
    NMAX = meta["NMAX"]
    WINSZ, NBLK, IDXCOLS = meta["WIN"], meta["NBLK"], meta["IDXCOLS"]
    groups, gathers, blocks = meta["groups"], meta["gathers"], meta["blocks"]
    grp_ch0 = meta["grp_ch0"]
    nchunk = meta["nchunk"]
    chunk_t0, chunk_t1 = meta["chunk_t0"], meta["chunk_t1"]
    chrows, chbase = meta["chrows"], meta["chbase"]
    GRP_ = meta["GRP"]
    f32 = mybir.dt.float32
    AT = mybir.ActivationFunctionType
    OP = mybir.AluOpType

    h1slice = nc.dram_tensor("h1slice", [NMAX, H], f32, kind="Internal")
    h2slice = nc.dram_tensor("h2slice", [NMAX, H], f32, kind="Internal")
    table1 = nc.dram_tensor("table1", [TOT, H], f32, kind="Internal")
    table2 = nc.dram_tensor("table2", [TOT, H], f32, kind="Internal")
    rg = [list(range(NC_CORES))]

    grp_nch = []
    for gi in range(len(groups)):
        c0 = grp_ch0[gi]
        c1 = grp_ch0[gi + 1] if gi + 1 < len(groups) else NBLK
        grp_nch.append(c1 - c0)

    first_blk = {}
    last_blk = {}
    for (t, w, chi) in blocks:
        if t not in first_blk:
            first_blk[t] = chi
        last_blk[t] = chi

    with ExitStack() as ctx:
        cpool = ctx.enter_context(tc.tile_pool(name="consts", bufs=1))
        plpool = ctx.enter_context(
            tc.tile_pool(name="ps_pool", bufs=1, space="PSUM"))

        w1_t = cpool.tile([DIN, H], f32, tag="w1")
        w2_t = cpool.tile([H, H], f32, tag="w2")
        wl_t = cpool.tile([H, C], f32, tag="wl")
        b1_t = cpool.tile([H, 1], f32, tag="b1")
        b2_t = cpool.tile([P, H], f32, tag="b2t")
        bl_t = cpool.tile([P, C], f32, tag="blt")
        iota_t = cpool.tile([P, P], f32, tag="iota")
        icnt_t = cpool.tile([P, 1], f32, tag="invcnt")
        gidx_t = cpool.tile([P, IDXCOLS], mybir.dt.int16, tag="gidx")
        dstloc_t = cpool.tile([P, NBLK], f32, tag="dstloc")
        nrm_t = cpool.tile([P, NBLK], f32, tag="nrm")
        bloc_t = cpool.tile([P, NT], f32, tag="batchloc")
        ident_t = cpool.tile([P, P], f32, tag="ident")

        for tile_, name in [
            (w1_t, "w1"), (w2_t, "w2"), (wl_t, "wl"), (b1_t, "b1"),
            (b2_t, "b2t"), (bl_t, "blt"), (iota_t, "iota"),
            (icnt_t, "invcnt"), (gidx_t, "gidx"), (dstloc_t, "dstloc"),
            (nrm_t, "nrm"), (bloc_t, "batchloc"),
        ]:
            nc.sync.dma_start(out=tile_[:], in_=ins[name][:])
        make_identity(nc, ident_t[:])

        pooled_ps = plpool.tile([P, H], f32, tag="pooled")

        # ---------------- phase A: h1 = x @ W1, chunked allgather -> table1
        with ExitStack() as actx:
            xpool = actx.enter_context(tc.tile_pool(name="xt", bufs=3))
            hpool = actx.enter_context(tc.tile_pool(name="hstage", bufs=4))
            pspool = actx.enter_context(
                tc.tile_pool(name="ps_a", bufs=3, space="PSUM"))
            for k in range(nchunk):
                for t in range(chunk_t0[k], chunk_t1[k]):
                    xt_t = xpool.tile([DIN, P], f32, tag="xt")
                    nc.sync.dma_start(out=xt_t[:],
                                      in_=ins["xt"][:, bass.ts(t, P)])
                    ps = pspool.tile([P, H], f32, tag="ps_a")
                    nc.tensor.matmul(out=ps[:], lhsT=xt_t[:], rhs=w1_t[:],
                                     start=True, stop=True)
                    h1s = hpool.tile([P, H], f32, tag="h1s")
                    nc.vector.tensor_copy(out=h1s[:], in_=ps[:])
                    nc.sync.dma_start(out=h1slice[bass.ts(t, P), :],
                                      in_=h1s[:])
                r0 = P * chunk_t0[k]
                nc.gpsimd.collective_compute(
                    "AllGather", OP.bypass, replica_groups=rg,
                    ins=[h1slice[r0:r0 + chrows[k], :]],
                    outs=[table1[chbase[k]:chbase[k + 1], :]],
                )

        # ---------------- message passing layers
        with ExitStack() as mctx:
            gbpool = mctx.enter_context(tc.tile_pool(name="gbuf", bufs=2))
            selpool = mctx.enter_context(tc.tile_pool(name="sel", bufs=8))
            mppool = mctx.enter_context(
                tc.tile_pool(name="ps_mp", bufs=GRP_, space="PSUM"))
            eppool = mctx.enter_context(tc.tile_pool(name="epil", bufs=4))
            h2pspool = mctx.enter_context(
                tc.tile_pool(name="ps_h2", bufs=1, space="PSUM"))

            def msgpass(layer, table):
                for gi, g in enumerate(groups):
                    nch = grp_nch[gi]
                    c0 = grp_ch0[gi]
                    gbuf = gbpool.tile([P, nch, H], f32, tag="gbuf")
                    for ga in gathers:
                        if ga["gi"] != gi:
                            continue
                        wlo = ga["w"] * WINSZ
                        whi = min(TOT, wlo + WINSZ)
                        cc0 = ga["chunk0"] - c0
                        ncw = ga["num"] // P
                        nc.gpsimd.dma_gather(
                            out_ap=gbuf[:, cc0:cc0 + ncw, :],
                            in_ap=table[wlo:whi, :],
                            idxs_ap=gidx_t[:, ga["chunk0"] * 8:
                                           ga["chunk0"] * 8 + ncw * 8],
                            num_idxs=ga["num"],
                            num_idxs_reg=ga["num"],
                            elem_size=H,
                            single_packet=False,
                        )
                    ps_of_tile = {}
                    for t in g:
                        shape = [H, P] if layer == 1 else [P, H]
                        ps_of_tile[t] = mppool.tile(shape, f32, tag="ps_mp",
                                                    name=f"psmp_{layer}_{t}")
                    for (t, w, chi) in blocks:
                        if not (c0 <= chi < c0 + nch):
                            continue
                        sel = selpool.tile([P, P], f32, tag="sel")
                        nc.vector.tensor_scalar(
                            out=sel[:], in0=iota_t[:],
                            scalar1=dstloc_t[:, chi:chi + 1],
                            scalar2=nrm_t[:, chi:chi + 1],
                            op0=OP.is_equal, op1=OP.mult,
                        )
                        ps = ps_of_tile[t]
                        st = first_blk[t] == chi
                        sp = last_blk[t] == chi
                        msg = gbuf[:, chi - c0, :]
                        if layer == 1:
                            nc.tensor.matmul(out=ps[:], lhsT=msg, rhs=sel[:],
                                             start=st, stop=sp)
                        else:
                            nc.tensor.matmul(out=ps[:], lhsT=sel[:], rhs=msg,
                                             start=st, stop=sp)
                        if not sp:
                            continue
                        if layer == 1:
                            o1T = eppool.tile([H, P], f32, tag="o1T")
                            nc.scalar.activation(out=o1T[:], in_=ps[:],
                                                 func=AT.Relu,
                                                 bias=b1_t[:, :1], scale=1.0)
                            h2ps = h2pspool.tile([P, H], f32, tag="h2ps")
                            nc.tensor.matmul(out=h2ps[:], lhsT=o1T[:],
                                             rhs=w2_t[:], start=True,
                                             stop=True)
                            h2s = eppool.tile([P, H], f32, tag="h2s")
                            nc.vector.tensor_copy(out=h2s[:], in_=h2ps[:])
                            nc.sync.dma_start(out=h2slice[bass.ts(t, P), :],
                                              in_=h2s[:])
                        else:
                            tmp = eppool.tile([P, H], f32, tag="o2tmp")
                            nc.vector.tensor_tensor(out=tmp[:], in0=ps[:],
                                                    in1=b2_t[:], op=OP.add)
                            o2 = eppool.tile([P, H], f32, tag="o2")
                            nc.scalar.activation(out=o2[:], in_=tmp[:],
                                                 func=AT.Relu)
                            ohg = selpool.tile([P, P], f32, tag="ohg")
                            nc.vector.tensor_scalar(
                                out=ohg[:], in0=iota_t[:],
                                scalar1=bloc_t[:, t:t + 1], scalar2=None,
                                op0=OP.is_equal,
                            )
                            nc.tensor.matmul(out=pooled_ps[:], lhsT=ohg[:],
                                             rhs=o2[:], start=(t == 0),
                                             stop=(t == NT - 1))
                    if layer == 1:
                        for k in range(nchunk):
                            if chunk_t1[k] - 1 in g:
                                r0 = P * chunk_t0[k]
                                nc.gpsimd.collective_compute(
                                    "AllGather", OP.bypass, replica_groups=rg,
                                    ins=[h2slice[r0:r0 + chrows[k], :]],
                                    outs=[table2[chbase[k]:chbase[k + 1], :]],
                                )

            msgpass(1, table1)
            msgpass(2, table2)

        # ---------------- phase C: pooled -> logits
        with ExitStack() as fctx:
            fpool = fctx.enter_context(tc.tile_pool(name="final", bufs=1))
            fps = fctx.enter_context(
                tc.tile_pool(name="ps_fin", bufs=2, space="PSUM"))
            pooled_s = fpool.tile([P, H], f32, tag="pooled_s")
            nc.vector.tensor_scalar(out=pooled_s[:], in0=pooled_ps[:],
                                    scalar1=icnt_t[:, :1], scalar2=None,
                                    op0=OP.mult)
            pT_ps = fps.tile([H, P], f32, tag="pT")
            nc.tensor.transpose(out=pT_ps[:], in_=pooled_s[:],
                                identity=ident_t[:])
            pT_s = fpool.tile([H, P], f32, tag="pT_s")
            nc.vector.tensor_copy(out=pT_s[:], in_=pT_ps[:])
            lg_ps = fps.tile([P, C], f32, tag="lg")
            nc.tensor.matmul(out=lg_ps[:], lhsT=pT_s[:], rhs=wl_t[:],
                             start=True, stop=True)
            lg_s = fpool.tile([P, C], f32, tag="lg_s")
            nc.vector.tensor_tensor(out=lg_s[:], in0=lg_ps[:], in1=bl_t[:],
                                    op=OP.add)
            nc.sync.dma_start(out=outs["logits"][:], in_=lg_s[:meta["GPC"], :])


# ------------------------------------------------------------------- runner

def _make_in_maps(meta, percore, winputs):
    in_maps = []
    for c in range(NC_CORES):
        m = {}
        for name in ["xt", "gidx", "dstloc", "nrm", "batchloc", "invcnt"]:
            m[name] = np.ascontiguousarray(percore[name][c])
        for name in ["w1", "w2", "wl", "b1", "b2t", "blt", "iota"]:
            m[name] = winputs[name]
        in_maps.append(m)
    return in_maps


def _build_program(meta, percore, winputs):
    import concourse.mybir as mybir
    import concourse.tile as tile
    from concourse import bacc

    f32 = mybir.dt.float32
    nc = bacc.Bacc("TRN2", target_bir_lowering=False, debug=False,
                   num_devices=NC_CORES)

    ins = {}
    for name in ["xt", "gidx", "dstloc", "nrm", "batchloc", "invcnt"]:
        arr = percore[name][0]
        ins[name] = nc.dram_tensor(
            name, list(arr.shape), mybir.dt.from_np(arr.dtype),
            kind="ExternalInput").ap()
    for name in ["w1", "w2", "wl", "b1", "b2t", "blt", "iota"]:
        arr = winputs[name]
        ins[name] = nc.dram_tensor(
            name, list(arr.shape), mybir.dt.from_np(arr.dtype),
            kind="ExternalInput").ap()
    out_ap = nc.dram_tensor("logits", [meta["GPC"], C], f32,
                            kind="ExternalOutput").ap()

    with tile.TileContext(nc) as tc:
        _build(tc, {"logits": out_ap}, ins, meta)
    nc.compile()
    return nc


def _run_on_hw(meta, percore, winputs, trace=False):
    from concourse.bass_utils import run_bass_kernel_spmd

    nc = _build_program(meta, percore, winputs)
    in_maps = _make_in_maps(meta, percore, winputs)
    res = run_bass_kernel_spmd(nc, in_maps, core_ids=list(range(NC_CORES)),
                               trace=trace)
    outs = [res.results[c]["logits"] for c in range(NC_CORES)]
    return np.concatenate(outs, axis=0), res


def kernel(x, edge_index, batch, W1, b1, W2, b2, Wl, bl):
    meta, percore = _preprocess(x, edge_index, batch, force_g=512)
    winputs = _weights_inputs(W1, b1, W2, b2, Wl, bl)
    out, _ = _run_on_hw(meta, percore, winputs, trace=False)
    return np.asarray(out, np.float32)
